# revision 13
# baseline (speedup 1.0000x reference)
"""Trainium2 Bass kernel for nn_NewAttention_55344948576827.

Math: reference computes
    v   = x @ W1.T                      (x: [B,S,E], W1: [E,E])
    att = w_pos @ v  (per head)         (w_pos[q,k] = c*exp(-0.5*(k-q)^2), [S,S])
    out = att @ W2.T
The positional weights are data independent, identical for every head and
channel, and decay below fp32 noise for |k-q| > 8 — so the [S,S] matmul is a
17-tap 1D convolution along S.  The conv acts on the S axis only and both
projections act on the E axis only, so they commute:
    out = conv_S(x) @ (W2 @ W1).T
One fused weight matrix, one conv.

Device mapping (per core):
  - conv_S as PE matmuls against constant banded matrices; with x tiles as
    the stationary operand this *also* transposes x (e lands on partitions),
    exactly the layout the main matmul needs.  x is shipped as OVERLAPPING
    128-row tiles at stride 112, so each 224-column conv block is exactly
    TWO matmuls (full-width BB with start=True, then BA accumulating into
    the low 128 columns) — no tiny K=16 leftovers.
  - WcT = W1.T @ W2.T computed on-device once per core, accumulated pair by
    pair as the (W1 chunk, W2T chunk) DMAs land, interleaved with conv
    block 0.  W2 is shipped pre-transposed from the host — a zero-FLOP
    relayout, like the halo prep.
  - main: out[s,f] = sum_e xcT[e,s] * WcT[e,f], 4 accumulating matmuls per
    128-row output tile, reading 128-col windows of a contiguous xcT buffer.

Sharding: B*S = 16384 rows split 8 ways -> 2048 rows/core (half a batch, so
the conv never crosses a core's slice except through an 8-row halo baked
into the shipped input).  No collectives.

Layout/perf notes:
  - Inputs shipped in [128, n, free] partition-major layout; few large DMAs
    (HWDGE issue costs ~0.6us each on the sync sequencer, serialized).
  - DMA issue order = arrival order: head_a (bands + weight pair 0), x tiles
    0-1, head_b (weight pairs 1-3), then the bulk x groups.  PE program
    order matches: WcT round 0, conv block 0, WcT rounds 1-3, then the
    steady conv/main pipeline.
  - Conv psum packs two e-chunks per [128,448] bank; one DVE copy per bank
    into the contiguous xcT_all buffer.
  - Output copies alternate DVE / ScalarE; output DMAs alternate both HWDGE
    rings; the last row-tile splits its copy+DMA in half across engines and
    rings to shorten the serial tail.
"""

import os
import threading
from contextlib import ExitStack

import numpy as np

import concourse.bass as bass
import concourse.tile as tile
from concourse import bacc, mybir
from concourse.bass_utils import run_bass_kernel_spmd

# ---------------------------------------------------------------- constants
B, S, E = 4, 4096, 512
N_CORES = 8
ROWS = (B * S) // N_CORES          # 2048 rows per core
R = 8                              # gaussian band radius (17 taps)
TS = 112                           # x tile stride (tiles overlap by 16)
N_XT = 19                          # x tiles; tile u = xext[112u : 112u+128]
XEXT = TS * (N_XT - 1) + 128       # 2144 padded rows of xext
BLK = 224                          # conv block columns (window = BLK+2R = 240)
N_BLK = 9                          # full conv blocks; + one 32-col tail block
X_GROUPS = (2, 6, 6, 5)            # x tiles per DMA batch

# per-partition column offsets inside the packed "head" tensors
BA_OFF = 0                         # BA  [128, 128]
BB_OFF = 128                       # BB  [128, 224]
WP_OFF = 352                       # weight pair 0: [2, 512]
HEADA_COLS = WP_OFF + 2 * E        # 1376
HEADB_COLS = 3 * 2 * E             # weight pairs 1-3

# matmul dtype: "bf16" (inputs quantized host-side, ~4e-3 rel err) or
# "f32r" (full fp32 data, relaxed-precision PE mode, ~2.5e-4 rel err, slower)
DTYPE = os.environ.get("KERNEL_DTYPE", "bf16")

_lock = threading.Lock()
_cache = {}


def _gauss(d):
    coef = 1.0 / np.sqrt(2.0 * np.pi)
    out = coef * np.exp(-0.5 * d.astype(np.float64) ** 2)
    out[np.abs(d) > R] = 0.0
    return out


def _band_mats():
    """BA[a, j] = g(a - 8 - j)  ([128, 128]; block-local x rows vs out cols)
       BB[b, j] = g(104 + b - j) for b >= 16 else 0  ([128, 224])."""
    a = np.arange(128)[:, None]
    j = np.arange(128)[None, :]
    BA = _gauss(a - R - j)
    j2 = np.arange(BLK)[None, :]
    BB = _gauss(104 + a - j2)
    BB[:16] = 0.0                  # rows covered by BA
    return BA.astype(np.float32), BB.astype(np.float32)


def _build(dtype_flag: str):
    mdt = {"f32r": mybir.dt.float32r, "bf16": mybir.dt.bfloat16}[dtype_flag]
    f32 = mybir.dt.float32

    nc = bacc.Bacc("TRN2", target_bir_lowering=False, debug=False,
                   num_devices=N_CORES)

    xd = nc.dram_tensor("x", [128, N_XT, E], mdt, kind="ExternalInput").ap()
    had = nc.dram_tensor("head_a", [128, HEADA_COLS], mdt,
                         kind="ExternalInput").ap()
    hbd = nc.dram_tensor("head_b", [128, HEADB_COLS], mdt,
                         kind="ExternalInput").ap()
    od = nc.dram_tensor("out", [ROWS, E], f32, kind="ExternalOutput").ap()

    with tile.TileContext(nc) as tc, ExitStack() as ctx:
        xp = ctx.enter_context(tc.tile_pool(name="xp", bufs=1))
        wp = ctx.enter_context(tc.tile_pool(name="wp", bufs=1))
        wctp = ctx.enter_context(tc.tile_pool(name="wctp", bufs=4))
        cvp = ctx.enter_context(tc.tile_pool(name="cvp", bufs=1))
        outp = ctx.enter_context(tc.tile_pool(name="outp", bufs=4))
        psA = ctx.enter_context(tc.tile_pool(name="psA", bufs=4, space="PSUM"))
        psB = ctx.enter_context(tc.tile_pool(name="psB", bufs=4, space="PSUM"))

        # ------------------------------------------------ input DMAs
        ha = wp.tile([128, HEADA_COLS], mdt, tag="ha")
        nc.sync.dma_start(out=ha[:], in_=had[:])

        xg = []

        def _load_xg(gi, off, gsz):
            g = xp.tile([128, gsz, E], mdt, tag=f"xg{gi}", name=f"xg{gi}")
            nc.sync.dma_start(out=g[:], in_=xd[:, off:off + gsz, :])
            xg.append((off, gsz, g))

        hb = wp.tile([128, HEADB_COLS], mdt, tag="hb")
        nc.sync.dma_start(out=hb[:], in_=hbd[:])

        _load_xg(0, 0, X_GROUPS[0])

        off = X_GROUPS[0]
        for gi, gsz in enumerate(X_GROUPS[1:], start=1):
            _load_xg(gi, off, gsz)
            off += gsz

        def xt(u):
            for off_, gsz_, g_ in xg:
                if off_ <= u < off_ + gsz_:
                    return g_[:, u - off_, :]
            raise IndexError(u)

        BA = ha[:, BA_OFF:BA_OFF + 128]
        BB = ha[:, BB_OFF:BB_OFF + BLK]

        def wpair(mi):        # -> (w1 chunk [128, E], w2T chunk [128, E])
            if mi == 0:
                base = ha[:, WP_OFF:WP_OFF + 2 * E]
            else:
                base = hb[:, 2 * E * (mi - 1):2 * E * mi]
            return base[:, 0:E], base[:, E:2 * E]

        # contiguous conv output: xcT_all[e_part, ei, s]
        xcT_all = cvp.tile([128, 4, ROWS], mdt, tag="xcT")

        def conv_block(t):
            # block t: out cols s in [224t, 224t+224), window x rows
            # [224t-8, 224t+232) = tile 2t (all) + tile 2t+1 (rows 16:128).
            # BB covers the window tail for all 224 cols (start=True), BA
            # accumulates the head into cols [0,128).
            for pi in range(2):
                pc = psA.tile([128, 2 * BLK], f32, tag="psA", name="psA_t")
                for sub in range(2):
                    ei = 2 * pi + sub
                    es = slice(128 * ei, 128 * ei + 128)
                    base = BLK * sub
                    nc.tensor.matmul(pc[:, base:base + BLK],
                                     xt(2 * t + 1)[:, es], BB,
                                     start=True, stop=False)
                    nc.tensor.matmul(pc[:, base:base + 128],
                                     xt(2 * t)[:, es], BA,
                                     start=False, stop=True)
                src = pc[:].rearrange("p (a b) -> p a b", a=2)
                nc.vector.tensor_copy(
                    xcT_all[:, 2 * pi:2 * pi + 2, BLK * t:BLK * t + BLK], src)

        def conv_tail():
            # out cols [2016, 2048): window rows [2008, 2064) all inside
            # tile 18; band values for rows >= 48 are zero.
            for pi in range(2):
                pc = psB.tile([128, 64], f32, tag="psB", name="psA9_t")
                for sub in range(2):
                    ei = 2 * pi + sub
                    es = slice(128 * ei, 128 * ei + 128)
                    nc.tensor.matmul(pc[:, 32 * sub:32 * sub + 32],
                                     xt(18)[:, es], BA[:, 0:32],
                                     start=True, stop=True)
                src = pc[:].rearrange("p (a b) -> p a b", a=2)
                nc.vector.tensor_copy(
                    xcT_all[:, 2 * pi:2 * pi + 2, 9 * BLK:ROWS], src)

        def main_tile(r, wcT):
            ot = outp.tile([128, E], f32, tag="ot", name=f"ot{r}")
            if r == 15:
                # tail: two independent psum banks so the half copies run
                # truly in parallel (DVE+ACT, different banks), then two
                # DMAs on both HWDGE rings — shortens the serial drain
                po1 = psB.tile([128, E], f32, tag="psB", name="psB_t1")
                po2 = psB.tile([128, E], f32, tag="psB", name="psB_t2")
                for ei in range(4):
                    xs = xcT_all[:, ei, 128 * r:128 * r + 128]
                    nc.tensor.matmul(po1[:, 0:256], xs, wcT[ei][:, 0:256],
                                     start=(ei == 0), stop=(ei == 3))
                for ei in range(4):
                    xs = xcT_all[:, ei, 128 * r:128 * r + 128]
                    nc.tensor.matmul(po2[:, 0:256], xs, wcT[ei][:, 256:512],
                                     start=(ei == 0), stop=(ei == 3))
                nc.vector.tensor_copy(ot[:, 0:256], po1[:, 0:256])
                nc.scalar.copy(ot[:, 256:512], po2[:, 0:256])
                nc.sync.dma_start(out=od[128 * r:128 * r + 128, 0:256],
                                  in_=ot[:, 0:256])
                nc.scalar.dma_start(out=od[128 * r:128 * r + 128, 256:512],
                                    in_=ot[:, 256:512])
            else:
                po = psB.tile([128, E], f32, tag="psB", name="psB_t")
                for ei in range(4):
                    nc.tensor.matmul(po[:],
                                     xcT_all[:, ei, 128 * r:128 * r + 128],
                                     wcT[ei][:],
                                     start=(ei == 0), stop=(ei == 3))
                if r % 2 == 0:
                    nc.vector.tensor_copy(ot[:], po[:])
                else:
                    nc.scalar.copy(ot[:], po[:])
                eng = nc.scalar if r % 2 == 0 else nc.sync
                eng.dma_start(out=od[128 * r:128 * r + 128, :], in_=ot[:])

        # ------------------ WcT[e,f] = sum_m W1[m,e] W2T[m,f], pair by pair
        wct_ps = [psB.tile([128, E], f32, tag="psB", name=f"wctps_{i}")
                  for i in range(4)]

        def wct_round(mi):
            w1c, w2c = wpair(mi)
            for ei in range(4):
                nc.tensor.matmul(wct_ps[ei][:],
                                 w1c[:, 128 * ei:128 * ei + 128], w2c[:],
                                 start=(mi == 0), stop=(mi == 3))

        # PE program order tracks DMA arrival order.
        wct_round(0)
        conv_block(0)
        wcT = [wctp.tile([128, E], mdt, tag="wcT", name=f"wcT_{i}")
               for i in range(4)]
        for mi in range(1, 4):
            wct_round(mi)
        for ei in range(4):
            nc.vector.tensor_copy(wcT[ei][:], wct_ps[ei][:])

        # steady pipeline: emit main tile r once its 128-col window is
        # covered by completed conv blocks (r+1)*128 <= (t+1)*224
        r_done = 0
        main_tile(0, wcT)
        r_done = 1
        for t in range(1, N_BLK):
            conv_block(t)
            while (r_done + 1) * 128 <= (t + 1) * BLK and r_done < 16:
                main_tile(r_done, wcT)
                r_done += 1
        conv_tail()
        while r_done < 16:
            main_tile(r_done, wcT)
            r_done += 1

    nc.compile()
    return nc


def _get_nc(dtype_flag: str):
    with _lock:
        if dtype_flag not in _cache:
            _cache[dtype_flag] = _build(dtype_flag)
        return _cache[dtype_flag]


def _np_dtype(dtype_flag: str):
    if dtype_flag == "bf16":
        import ml_dtypes
        return ml_dtypes.bfloat16
    return np.float32


def _part_major(a: np.ndarray) -> np.ndarray:
    """[n*128, free] -> [128, n, free] (partition-major DMA layout)."""
    n = a.shape[0] // 128
    return np.ascontiguousarray(
        a.reshape(n, 128, a.shape[1]).transpose(1, 0, 2))


def make_in_maps(x: np.ndarray, w1: np.ndarray, w2: np.ndarray,
                 dtype_flag: str):
    ndt = _np_dtype(dtype_flag)
    BA, BB = _band_mats()
    w1s = _part_major(w1.astype(ndt))                    # [128, 4, E]
    w2s = _part_major(np.ascontiguousarray(w2.T).astype(ndt))

    head_a = np.zeros((128, HEADA_COLS), dtype=ndt)
    head_a[:, BA_OFF:BA_OFF + 128] = BA.astype(ndt)
    head_a[:, BB_OFF:BB_OFF + BLK] = BB.astype(ndt)
    head_a[:, WP_OFF:WP_OFF + E] = w1s[:, 0, :]
    head_a[:, WP_OFF + E:WP_OFF + 2 * E] = w2s[:, 0, :]
    head_b = np.zeros((128, HEADB_COLS), dtype=ndt)
    for mi in range(1, 4):
        head_b[:, 2 * E * (mi - 1):2 * E * (mi - 1) + E] = w1s[:, mi, :]
        head_b[:, 2 * E * (mi - 1) + E:2 * E * mi] = w2s[:, mi, :]

    halves = S // 2
    tix = (TS * np.arange(N_XT))[:, None] + np.arange(128)[None, :]
    in_maps = []
    for c in range(N_CORES):
        b, half = divmod(c, 2)
        s0 = half * halves
        s1 = s0 + halves
        xext = np.zeros((XEXT, E), dtype=ndt)
        xext[R:R + ROWS] = x[b, s0:s1].astype(ndt)
        if s0 > 0:
            xext[0:R] = x[b, s0 - R:s0].astype(ndt)
        if s1 < S:
            xext[R + ROWS:R + ROWS + R] = x[b, s1:s1 + R].astype(ndt)
        xtiles = np.ascontiguousarray(xext[tix].transpose(1, 0, 2))
        in_maps.append({"x": xtiles, "head_a": head_a, "head_b": head_b})
    return in_maps


def kernel(inputs: np.ndarray, input_weights: np.ndarray,
           output_weight: np.ndarray) -> np.ndarray:
    x = np.ascontiguousarray(np.asarray(inputs, dtype=np.float32))
    w1 = np.asarray(input_weights, dtype=np.float32)
    w2 = np.asarray(output_weight, dtype=np.float32)
    assert x.shape == (B, S, E) and w1.shape == (E, E) and w2.shape == (E, E)

    nc = _get_nc(DTYPE)
    in_maps = make_in_maps(x, w1, w2, DTYPE)
    res = run_bass_kernel_spmd(nc, in_maps, core_ids=list(range(N_CORES)))

    halves = S // 2
    out = np.empty((B, S, E), dtype=np.float32)
    for c in range(N_CORES):
        b, half = divmod(c, 2)
        s0 = half * halves
        out[b, s0:s0 + halves] = res.results[c]["out"]
    return out
